# revision 19
# baseline (speedup 1.0000x reference)
"""Trainium2 Bass kernel: 1D box filter (window 17, zero-padded) along seq.

out[b, t, d] = (1/17) * sum_{i=-8..8} x[b, t+i, d]   (zero-padded in t)

Full input [8, 8192, 1024] f32. Batch dim sharded across 8 NeuronCores
(data-parallel, no cross-core communication).

Per-core algorithm: the window sum along seq is a banded matmul. Put 128
consecutive input seq rows on SBUF partitions (natural, fully-coalesced DMA
layout), multiply by a constant banded matrix A [K=128, M=112] with
A[k, m] = 1/17 for m <= k <= m+16, so PSUM[m, d] = window sum for output row
t0+m from input rows t0-8+k. 112 output rows per 128-row (halo +-8) input
tile; D=1024 split into two N=512 matmuls (PSUM bank limit). ScalarE
evacuates PSUM -> SBUF, DMA stores. Groups are batched 4-at-a-time into
supergroups (~2MB per HBM DMA, 5 SBUF bufs for deep overlap); input DMAs
ride the SP HWDGE ring, output DMAs the ACT ring so stores never
head-of-line-block loads. Cost-model (TimelineSim) predicted exec time:
~204 us/core, DMA-bandwidth-bound (72MB HBM traffic at ~360GB/s/core;
PE 126us and ScalarE ~60us fully hidden).
"""

import numpy as np

import orjson

import concourse.bass as bass
import concourse.mybir as mybir
from concourse.bass_utils import run_bass_kernel_spmd
from concourse.tile import TileContext

# The installed walrus rejects >2 embedded sync waits on one instruction
# ("Too many sync wait commands"), while this Tile version freely packs 3+
# waits onto engine instructions (and every live semaphore onto the kernel
# tail drain). Post-process the serialized BIR: excess waits move onto
# standalone EventSemaphore instructions injected just before the owning
# instruction on the same engine queue, which preserves semantics (all
# waits still happen-before the instruction).
_WAIT_LIMIT_DEFAULT = 1
# EventSemaphore and Drain accept 2 embedded waits; LDWEIGHTS/DMA take 1.
_WAIT_LIMIT_BY_OPCODE = {"EventSemaphore": 2}
_EVSEM_WAITS = 2  # waits per injected EventSemaphore


def _split_sync_waits(bir_bytes: bytes) -> bytes:
    bir = orjson.loads(bir_bytes)
    ctr = 0
    for fn in bir.get("functions", []):
        for bb in fn.get("blocks", []):
            insts = bb.get("instructions")
            if not insts:
                continue
            out = []
            changed = False
            for ins in insts:
                si = ins.get("sync_info")
                ow = (si or {}).get("on_wait") or []
                limit = _WAIT_LIMIT_BY_OPCODE.get(
                    ins.get("opcode"), _WAIT_LIMIT_DEFAULT
                )
                if len(ow) > limit:
                    extra, keep = ow[:-limit] if limit else ow, ow[-limit:] if limit else []
                    for c0 in range(0, len(extra), _EVSEM_WAITS):
                        ctr += 1
                        out.append(
                            {
                                "debug": ins.get("debug", 0),
                                "engine": ins["engine"],
                                "ins": [],
                                "outs": [],
                                "name": f"wsplit-{ctr}-{ins['name']}",
                                "opcode": "EventSemaphore",
                                "sync_info": {
                                    "on_update": [],
                                    "on_wait": extra[c0 : c0 + _EVSEM_WAITS],
                                },
                            }
                        )
                    si["on_wait"] = keep
                    changed = True
                out.append(ins)
            if changed:
                bb["instructions"] = out
    return orjson.dumps(bir)


class WaitSplitBass(bass.Bass):
    def to_json_bytes(self) -> bytes:
        return _split_sync_waits(super().to_json_bytes())

W = 8            # half window
WIN = 2 * W + 1  # 17
S = 8192         # seq len per core
D = 1024         # feature dim
B = 8            # batch == number of cores
M = 112          # output rows per matmul group (128 - 2*W)
K = 128          # input rows per group (partition dim)
N_HALF = 512     # matmul moving free dim (one PSUM bank of fp32)

F32 = mybir.dt.float32


def make_band() -> np.ndarray:
    """A[k, m] = 1/17 if m <= k <= m+16 else 0, shape [128, 112] fp32."""
    a = np.zeros((K, M), dtype=np.float32)
    for m in range(M):
        a[m : m + WIN, m] = 1.0 / WIN
    return a


def build_program(
    do_mm: bool = True,
    do_copy: bool = True,
    do_in: bool = True,
    do_out: bool = True,
    sg: int = 4,
    io_bufs: int = 5,
    out_dma_on_act: bool = True,
) -> bass.Bass:
    assert 72 % sg == 0
    nsg = 72 // sg
    nc = WaitSplitBass("TRN2", target_bir_lowering=False, debug=False)
    x = nc.dram_tensor("x", [S, D], F32, kind="ExternalInput")
    band = nc.dram_tensor("band", [K, M], F32, kind="ExternalInput")
    y = nc.dram_tensor("y", [S, D], F32, kind="ExternalOutput")

    with TileContext(nc) as tc:
        with (
            tc.tile_pool(name="const", bufs=1) as cpool,
            tc.tile_pool(name="io", bufs=io_bufs) as iopool,
            tc.tile_pool(name="psum", bufs=4, space="PSUM") as ppool,
        ):
            band_t = cpool.tile([K, M], F32)
            nc.sync.dma_start(out=band_t, in_=band.ap())

            def group_mms(psum_t, rhs2d, m_rows, k_rows):
                # psum_t: [m_rows, 1024] PSUM; rhs2d: [k_rows, 1024] SBUF
                if not do_mm:
                    return
                for h in range(2):
                    nc.tensor.matmul(
                        psum_t[:m_rows, h * N_HALF : (h + 1) * N_HALF],
                        band_t[:k_rows, :m_rows],
                        rhs2d[:k_rows, h * N_HALF : (h + 1) * N_HALF],
                        start=True,
                        stop=True,
                    )

            # ---- group 0: out rows [0, 112), input rows [-8, 120) ----
            g0_t = iopool.tile([K, D], F32, bufs=1)
            nc.any.memset(g0_t, 0.0)
            if do_in:
                nc.sync.dma_start(out=g0_t[W:K, :], in_=x.ap()[0 : K - W, :])
            g0_ps = ppool.tile([M, 2 * N_HALF], F32, tag="ps")
            group_mms(g0_ps, g0_t, M, K)
            g0_out = iopool.tile([M, D], F32, bufs=1)
            if do_copy:
                nc.scalar.copy(g0_out, g0_ps)
            if do_out:
                nc.sync.dma_start(out=y.ap()[0:M, :], in_=g0_out)

            # ---- supergroups: groups 1..72, out rows [112, 8176) ----
            out_dma_eng = nc.scalar if out_dma_on_act else nc.sync
            for s in range(nsg):
                g0s = 1 + sg * s
                base_in = (M * g0s - W) * D
                in_sg = iopool.tile([K, sg, D], F32)
                if do_in:
                    nc.sync.dma_start(
                        out=in_sg,
                        in_=bass.AP(x, base_in, [[D, K], [M * D, sg], [1, D]]),
                    )
                out_sg = iopool.tile([M, sg, D], F32)
                for j in range(sg):
                    ps = ppool.tile([M, 2 * N_HALF], F32, tag="ps")
                    group_mms(ps, in_sg[:, j, :], M, K)
                    if do_copy:
                        nc.scalar.copy(out_sg[:, j, :], ps)
                if do_out:
                    out_dma_eng.dma_start(
                        out=bass.AP(y, M * g0s * D, [[D, M], [M * D, sg], [1, D]]),
                        in_=out_sg,
                    )

            # ---- tail group: out rows [8176, 8192), input rows [8168, 8200) ----
            tail_rows = S - 73 * M           # 16
            tk = tail_rows + 2 * W           # 32 partitions
            tv = S - (73 * M - W)            # 24 valid input rows
            tail_t = iopool.tile([tk, D], F32, bufs=1)
            nc.any.memset(tail_t, 0.0)
            if do_in:
                nc.sync.dma_start(out=tail_t[0:tv, :], in_=x.ap()[S - tv : S, :])
            tail_ps = ppool.tile([M, 2 * N_HALF], F32, tag="ps")
            group_mms(tail_ps, tail_t, tail_rows, tk)
            tail_out = iopool.tile([tail_rows, D], F32, bufs=1)
            if do_copy:
                nc.scalar.copy(tail_out, tail_ps[:tail_rows, :])
            if do_out:
                nc.sync.dma_start(out=y.ap()[S - tail_rows : S, :], in_=tail_out)

    return nc


_CACHE: dict[str, bass.Bass] = {}


def get_program() -> bass.Bass:
    if "nc" not in _CACHE:
        _CACHE["nc"] = build_program()
    return _CACHE["nc"]


def make_in_maps(inputs: np.ndarray) -> list[dict[str, np.ndarray]]:
    band = make_band()
    return [{"x": inputs[b], "band": band} for b in range(B)]


def kernel(inputs) -> np.ndarray:
    inputs = np.ascontiguousarray(np.asarray(inputs), dtype=np.float32)
    assert inputs.shape == (B, S, D), inputs.shape
    nc = get_program()
    in_maps = make_in_maps(inputs)
    try:
        res = run_bass_kernel_spmd(nc, in_maps, list(range(B)))
    except Exception:
        # transient axon terminal failures have been observed; retry once
        res = run_bass_kernel_spmd(nc, in_maps, list(range(B)))
    return np.stack([res.results[b]["y"] for b in range(B)], axis=0)
